# revision 11
# baseline (speedup 1.0000x reference)
"""CLinear (4-bit group-quantized linear) Trainium2 kernel.

y = x @ dequant(packed, inv_scale, mn).T + bias

Full shapes: x [4096, 4096] f32, packed [11008, 64, 32] u8,
mn/inv_scale [11008, 64, 1] f32, bias [11008] f32 -> out [4096, 11008] f32.

Strategy (column-parallel over out_features across 8 cores):
  - O padded 11008 -> 11264 = 8 * 1408; each core owns an o-slice of 1408.
  - Host does *layout only*: transpose x -> x^T (bf16, tiled so each
    token-tile's DMA reads are 8KB-contiguous per partition), re-packs the
    4-bit nibbles so the device unpacks directly into W^T [k, o] k-tiles,
    and transposes the per-group scale/offset tensors.
  - Device: for each 128-row k-tile, one fused DVE scalar_tensor_tensor
    (nibble-extract via bitwise_and, multiply by group scale) + one
    tensor_tensor add of the group offset produces a resident bf16
    W^T tile.  The hi-nibble path uses scale/16 so no shift is needed.
  - Matmul: out-tile [128 tokens, <=512 o] PSUM accumulated over 32
    k-tiles; lhsT = x^T tile (stationary), rhs = W^T tile.  Bias is
    added during the PSUM->SBUF copy (DVE tensor_tensor add with a
    partition-broadcast bias tile).
"""

from contextlib import ExitStack

import ml_dtypes
import numpy as np

O_FULL, I_DIM = 11008, 4096
TOKENS = 4096
GS = 64
NG = I_DIM // GS
N_CORES = 8
O_PAD = 11264  # 8 * 1408 = 88 * 128
O_C = O_PAD // N_CORES

BF16NP = ml_dtypes.bfloat16


def o_chunks_for(o_c, chunk=512):
    chunks = []
    off = 0
    while off < o_c:
        ln = min(chunk, o_c - off)
        chunks.append((off, ln))
        off += ln
    return chunks


def build_bass(T=TOKENS, K=I_DIM, o_c=O_C, chunk=512, n_devices=N_CORES):
    import concourse.bass as bass
    import concourse.tile as tile
    from concourse import bacc, mybir

    BF16, F32, U8 = mybir.dt.bfloat16, mybir.dt.float32, mybir.dt.uint8
    Alu = mybir.AluOpType

    nkt = K // 128   # k-tiles of 128
    nbt = K // 256   # packed byte tiles (each yields 2 k-tiles)
    nti = T // 128   # token tiles
    ngc = K // GS    # groups along K
    chunks = o_chunks_for(o_c, chunk)

    nc = bacc.Bacc(
        "TRN2",
        target_bir_lowering=False,
        debug=False,
        enable_asserts=False,
        num_devices=n_devices,
    )
    xt = nc.dram_tensor("xt", [T, K], BF16, kind="ExternalInput").ap()
    pk = nc.dram_tensor("pk", [K // 2, o_c], U8, kind="ExternalInput").ap()
    # scale/offset pre-expanded on host to one row per k (scale/16 on hi k-tiles)
    sf = nc.dram_tensor("sf", [K, o_c], F32, kind="ExternalInput").ap()
    mf = nc.dram_tensor("mf", [K, o_c], F32, kind="ExternalInput").ap()
    bs = nc.dram_tensor("bs", [1, o_c], F32, kind="ExternalInput").ap()    # bias
    out = nc.dram_tensor("out", [T, o_c], F32, kind="ExternalOutput").ap()

    with ExitStack() as ctx:
        tc = ctx.enter_context(tile.TileContext(nc))
        wpool = ctx.enter_context(tc.tile_pool(name="wpool", bufs=nkt))
        bpool = ctx.enter_context(tc.tile_pool(name="bpool", bufs=3))
        spool = ctx.enter_context(tc.tile_pool(name="spool", bufs=3))
        tpool = ctx.enter_context(tc.tile_pool(name="tpool", bufs=2))
        xpool = ctx.enter_context(tc.tile_pool(name="xpool", bufs=3))
        pspool = ctx.enter_context(tc.tile_pool(name="pspool", bufs=8, space="PSUM"))
        opool = ctx.enter_context(tc.tile_pool(name="opool", bufs=6))
        singles = ctx.enter_context(tc.tile_pool(name="singles", bufs=1))

        # bias broadcast to all 128 partitions, once
        bias_sb = singles.tile([128, o_c], F32)
        nc.gpsimd.dma_start(
            out=bias_sb,
            in_=bass.AP(tensor=bs.tensor, offset=0, ap=[[0, 128], [1, o_c]]),
        )

        # ---- dequant: build resident W^T bf16 k-tiles ----
        wtiles = []
        for bt in range(nbt):
            pkt = bpool.tile([128, o_c], U8, tag="pkt")
            nc.sync.dma_start(out=pkt, in_=pk[bt * 128 : (bt + 1) * 128, :])
            for half in range(2):
                kt = 2 * bt + half
                s_exp = spool.tile([128, o_c], F32, tag="s_exp")
                m_exp = spool.tile([128, o_c], F32, tag="m_exp")
                nc.sync.dma_start(out=s_exp, in_=sf[kt * 128 : (kt + 1) * 128, :])
                nc.sync.dma_start(out=m_exp, in_=mf[kt * 128 : (kt + 1) * 128, :])
                msk = 0xF0 if half == 0 else 0x0F
                nib = tpool.tile([128, o_c], U8, tag="nib")
                nc.vector.tensor_scalar(
                    out=nib, in0=pkt, scalar1=msk, scalar2=None, op0=Alu.bitwise_and
                )
                tmp = tpool.tile([128, o_c], F32, tag="tmp")
                nc.vector.tensor_tensor(out=tmp, in0=nib, in1=s_exp, op=Alu.mult)
                w = wpool.tile([128, o_c], BF16, name=f"w{kt}", tag="w")
                nc.vector.tensor_add(w, tmp, m_exp)
                wtiles.append(w)

        # ---- matmul: out[ti, oc] = sum_kt xT[kt, ti].T @ WT[kt][:, oc] ----
        for ti in range(nti):
            xt_t = xpool.tile([128, nkt, 128], BF16, tag="xt_t")
            nc.sync.dma_start(out=xt_t, in_=xt[ti * 128 : (ti + 1) * 128, :])
            pss = []
            for o_off, o_len in chunks:
                ps = pspool.tile([128, o_len], F32, tag="ps")
                pss.append((ps, o_off, o_len))
            for kt in range(nkt):
                for ps, o_off, o_len in pss:
                    nc.tensor.matmul(
                        ps,
                        lhsT=xt_t[:, kt, :],
                        rhs=wtiles[kt][:, o_off : o_off + o_len],
                        start=(kt == 0),
                        stop=(kt == nkt - 1),
                    )
            for ps, o_off, o_len in pss:
                osb = opool.tile([128, o_len], F32, tag="osb")
                nc.vector.tensor_add(osb, ps, bias_sb[:, o_off : o_off + o_len])
                nc.sync.dma_start(
                    out=out[ti * 128 : (ti + 1) * 128, o_off : o_off + o_len],
                    in_=osb,
                )

    nc.compile()
    return nc


def host_prep(x, packed, mn, inv_scale, bias, o_pad=O_PAD, n_cores=N_CORES):
    """Pure layout transforms; returns per-core input maps."""
    x = np.asarray(x, dtype=np.float32)
    packed = np.asarray(packed, dtype=np.uint8)
    mn = np.asarray(mn, dtype=np.float32)
    inv_scale = np.asarray(inv_scale, dtype=np.float32)
    bias = np.asarray(bias, dtype=np.float32)

    T, K = x.shape
    O = packed.shape[0]
    ng = packed.shape[1]
    o_c = o_pad // n_cores
    nkt, nti, nbt = K // 128, T // 128, K // 256

    # x^T, bf16, rearranged so token-tile ti occupies rows [128*ti, 128*(ti+1))
    # with per-partition-contiguous K (each DMA row is a full 8KB K-run).
    xT = x.T.astype(BF16NP)  # [K, T]
    xh = np.ascontiguousarray(
        xT.reshape(nkt, 128, nti, 128).transpose(2, 1, 0, 3).reshape(T, K)
    )
    # row (ti*128 + p), col (kt*128 + t) = xT[kt*128 + p, ti*128 + t]

    # unpack nibbles -> q [O, K], transpose, pad, re-pack so that byte tile bt
    # unpacks into k-tiles 2bt (hi nibble) and 2bt+1 (lo nibble).
    hi = packed >> 4
    lo = packed & 0x0F
    q = np.concatenate([hi, lo], axis=2).reshape(O, K)  # q[o, g*64+j]
    qT = np.zeros((K, o_pad), dtype=np.uint8)
    qT[:, :O] = q.T
    q4 = qT.reshape(nbt, 2, 128, o_pad)
    pk = ((q4[:, 0] << 4) | q4[:, 1]).reshape(K // 2, o_pad)

    sT = np.zeros((ng, o_pad), dtype=np.float32)
    sT[:, :O] = inv_scale[:, :, 0].T
    mT = np.zeros((ng, o_pad), dtype=np.float32)
    mT[:, :O] = mn[:, :, 0].T
    # expand to one row per k; hi k-tiles (even kt) get scale/16 because the
    # nibble arrives as (byte & 0xF0) = 16*q.
    s_full = np.repeat(sT, GS, axis=0)  # [K, o_pad], row k -> group k//64
    m_full = np.repeat(mT, GS, axis=0)
    ktile_of_k = np.arange(K) // 128
    s_full[ktile_of_k % 2 == 0] /= 16.0
    bsr = np.zeros((1, o_pad), dtype=np.float32)
    bsr[0, :O] = bias

    in_maps = []
    for c in range(n_cores):
        s = slice(c * o_c, (c + 1) * o_c)
        in_maps.append(
            {
                "xt": xh,
                "pk": np.ascontiguousarray(pk[:, s]),
                "sf": np.ascontiguousarray(s_full[:, s]),
                "mf": np.ascontiguousarray(m_full[:, s]),
                "bs": np.ascontiguousarray(bsr[:, s]),
            }
        )
    return in_maps


def kernel(x, packed, mn, inv_scale, bias):
    from concourse.bass_utils import run_bass_kernel_spmd

    in_maps = host_prep(x, packed, mn, inv_scale, bias)
    nc = build_bass()
    res = run_bass_kernel_spmd(nc, in_maps, core_ids=list(range(N_CORES)))
    outs = [r["out"] for r in res.results]
    full = np.concatenate(outs, axis=1)[:, :O_FULL]
    return np.ascontiguousarray(full.astype(np.float32))


# revision 14
# speedup vs baseline: 72.1604x; 72.1604x over previous
"""CLinear (4-bit group-quantized linear) Trainium2 kernel.

y = x @ dequant(packed, inv_scale, mn).T + bias

Full shapes: x [4096, 4096] f32, packed [11008, 64, 32] u8,
mn/inv_scale [11008, 64, 1] f32, bias [11008] f32 -> out [4096, 11008] f32.

Strategy (column-parallel over out_features across 8 cores):
  - O padded 11008 -> 11264 = 8 * 1408; each core owns an o-slice of 1408.
  - Host does *layout only*: transpose x -> x^T (bf16, tiled so each
    token-tile's DMA reads are 8KB-contiguous per partition), re-packs the
    4-bit nibbles so the device unpacks directly into W^T [k, o] k-tiles,
    and transposes the per-group scale/offset tensors.
  - Device: for each 128-row k-tile, one fused DVE scalar_tensor_tensor
    (nibble-extract via bitwise_and, multiply by group scale) + one
    tensor_tensor add of the group offset produces a resident bf16
    W^T tile.  The hi-nibble path uses scale/16 so no shift is needed.
  - Matmul: out-tile [128 tokens, <=512 o] PSUM accumulated over 32
    k-tiles; lhsT = x^T tile (stationary), rhs = W^T tile.  Bias is
    added during the PSUM->SBUF copy (DVE tensor_tensor add with a
    partition-broadcast bias tile).
"""

from contextlib import ExitStack

import ml_dtypes
import numpy as np

O_FULL, I_DIM = 11008, 4096
TOKENS = 4096
GS = 64
NG = I_DIM // GS
N_CORES = 8
O_PAD = 11264  # 8 * 1408 = 88 * 128
O_C = O_PAD // N_CORES

BF16NP = ml_dtypes.bfloat16


def o_chunks_for(o_c, chunk=512):
    chunks = []
    off = 0
    while off < o_c:
        ln = min(chunk, o_c - off)
        chunks.append((off, ln))
        off += ln
    return chunks


def build_bass(T=TOKENS, K=I_DIM, o_c=O_C, chunk=512, n_devices=N_CORES, reps=1):
    import concourse.bass as bass
    import concourse.tile as tile
    from concourse import bacc, mybir

    BF16, F32, U8 = mybir.dt.bfloat16, mybir.dt.float32, mybir.dt.uint8
    Alu = mybir.AluOpType

    nkt = K // 128   # k-tiles of 128
    nbt = K // 256   # packed byte tiles (each yields 2 k-tiles)
    nti = T // 128   # token tiles
    ngc = K // GS    # groups along K
    chunks = o_chunks_for(o_c, chunk)

    nc = bacc.Bacc(
        "TRN2",
        target_bir_lowering=False,
        debug=False,
        enable_asserts=False,
        num_devices=n_devices,
    )
    xt = nc.dram_tensor("xt", [T, K], BF16, kind="ExternalInput").ap()
    pk = nc.dram_tensor("pk", [K // 2, o_c], U8, kind="ExternalInput").ap()
    # scale/offset pre-expanded on host to one row per k (scale/16 on hi k-tiles)
    sf = nc.dram_tensor("sf", [K, o_c], F32, kind="ExternalInput").ap()
    mf = nc.dram_tensor("mf", [K, o_c], F32, kind="ExternalInput").ap()
    bs = nc.dram_tensor("bs", [1, o_c], F32, kind="ExternalInput").ap()    # bias
    out = nc.dram_tensor("out", [T, o_c], F32, kind="ExternalOutput").ap()

    with ExitStack() as ctx:
        tc = ctx.enter_context(tile.TileContext(nc))
        wpool = ctx.enter_context(tc.tile_pool(name="wpool", bufs=nkt))
        bpool = ctx.enter_context(tc.tile_pool(name="bpool", bufs=3))
        spool = ctx.enter_context(tc.tile_pool(name="spool", bufs=3))
        tpool = ctx.enter_context(tc.tile_pool(name="tpool", bufs=2))
        xpool = ctx.enter_context(tc.tile_pool(name="xpool", bufs=3))
        pspool = ctx.enter_context(tc.tile_pool(name="pspool", bufs=8, space="PSUM"))
        opool = ctx.enter_context(tc.tile_pool(name="opool", bufs=6))
        singles = ctx.enter_context(tc.tile_pool(name="singles", bufs=1))

        # bias broadcast to all 128 partitions, once
        bias_sb = singles.tile([128, o_c], F32)
        nc.gpsimd.dma_start(
            out=bias_sb,
            in_=bass.AP(tensor=bs.tensor, offset=0, ap=[[0, 128], [1, o_c]]),
        )

        for rep in range(reps):
            # ---- dequant: build resident W^T bf16 k-tiles ----
            wtiles = []
            for bt in range(nbt):
                pkt = bpool.tile([128, o_c], U8, tag="pkt")
                nc.sync.dma_start(out=pkt, in_=pk[bt * 128 : (bt + 1) * 128, :])
                for half in range(2):
                    kt = 2 * bt + half
                    s_exp = spool.tile([128, o_c], F32, tag="s_exp")
                    m_exp = spool.tile([128, o_c], F32, tag="m_exp")
                    nc.sync.dma_start(out=s_exp, in_=sf[kt * 128 : (kt + 1) * 128, :])
                    nc.sync.dma_start(out=m_exp, in_=mf[kt * 128 : (kt + 1) * 128, :])
                    msk = 0xF0 if half == 0 else 0x0F
                    nib = tpool.tile([128, o_c], U8, tag="nib")
                    nc.vector.tensor_scalar(
                        out=nib, in0=pkt, scalar1=msk, scalar2=None, op0=Alu.bitwise_and
                    )
                    tmp = tpool.tile([128, o_c], F32, tag="tmp")
                    nc.vector.tensor_tensor(out=tmp, in0=nib, in1=s_exp, op=Alu.mult)
                    w = wpool.tile([128, o_c], BF16, name=f"w{kt}", tag="w")
                    nc.vector.tensor_add(w, tmp, m_exp)
                    wtiles.append(w)

            # ---- matmul: out[ti, oc] = sum_kt xT[kt, ti].T @ WT[kt][:, oc] ----
            for ti in range(nti):
                xt_t = xpool.tile([128, nkt, 128], BF16, tag="xt_t")
                nc.sync.dma_start(out=xt_t, in_=xt[ti * 128 : (ti + 1) * 128, :])
                pss = []
                for o_off, o_len in chunks:
                    ps = pspool.tile([128, o_len], F32, tag="ps")
                    pss.append((ps, o_off, o_len))
                for kt in range(nkt):
                    for ps, o_off, o_len in pss:
                        nc.tensor.matmul(
                            ps,
                            lhsT=xt_t[:, kt, :],
                            rhs=wtiles[kt][:, o_off : o_off + o_len],
                            start=(kt == 0),
                            stop=(kt == nkt - 1),
                        )
                for ps, o_off, o_len in pss:
                    osb = opool.tile([128, o_len], F32, tag="osb")
                    nc.vector.tensor_add(osb, ps, bias_sb[:, o_off : o_off + o_len])
                    nc.sync.dma_start(
                        out=out[ti * 128 : (ti + 1) * 128, o_off : o_off + o_len],
                        in_=osb,
                    )

    nc.compile()
    return nc


def host_prep(x, packed, mn, inv_scale, bias, o_pad=O_PAD, n_cores=N_CORES):
    """Pure layout transforms; returns per-core input maps."""
    x = np.asarray(x, dtype=np.float32)
    packed = np.asarray(packed, dtype=np.uint8)
    mn = np.asarray(mn, dtype=np.float32)
    inv_scale = np.asarray(inv_scale, dtype=np.float32)
    bias = np.asarray(bias, dtype=np.float32)

    T, K = x.shape
    O = packed.shape[0]
    ng = packed.shape[1]
    o_c = o_pad // n_cores
    nkt, nti, nbt = K // 128, T // 128, K // 256

    # x^T, bf16, rearranged so token-tile ti occupies rows [128*ti, 128*(ti+1))
    # with per-partition-contiguous K (each DMA row is a full 8KB K-run).
    xT = x.T.astype(BF16NP)  # [K, T]
    xh = np.ascontiguousarray(
        xT.reshape(nkt, 128, nti, 128).transpose(2, 1, 0, 3).reshape(T, K)
    )
    # row (ti*128 + p), col (kt*128 + t) = xT[kt*128 + p, ti*128 + t]

    # unpack nibbles -> q [O, K], transpose, pad, re-pack so that byte tile bt
    # unpacks into k-tiles 2bt (hi nibble) and 2bt+1 (lo nibble).
    hi = packed >> 4
    lo = packed & 0x0F
    q = np.concatenate([hi, lo], axis=2).reshape(O, K)  # q[o, g*64+j]
    qT = np.zeros((K, o_pad), dtype=np.uint8)
    qT[:, :O] = q.T
    q4 = qT.reshape(nbt, 2, 128, o_pad)
    pk = ((q4[:, 0] << 4) | q4[:, 1]).reshape(K // 2, o_pad)

    sT = np.zeros((ng, o_pad), dtype=np.float32)
    sT[:, :O] = inv_scale[:, :, 0].T
    mT = np.zeros((ng, o_pad), dtype=np.float32)
    mT[:, :O] = mn[:, :, 0].T
    # expand to one row per k; hi k-tiles (even kt) get scale/16 because the
    # nibble arrives as (byte & 0xF0) = 16*q.
    s_full = np.repeat(sT, GS, axis=0)  # [K, o_pad], row k -> group k//64
    m_full = np.repeat(mT, GS, axis=0)
    ktile_of_k = np.arange(K) // 128
    s_full[ktile_of_k % 2 == 0] /= 16.0
    bsr = np.zeros((1, o_pad), dtype=np.float32)
    bsr[0, :O] = bias

    in_maps = []
    for c in range(n_cores):
        s = slice(c * o_c, (c + 1) * o_c)
        in_maps.append(
            {
                "xt": xh,
                "pk": np.ascontiguousarray(pk[:, s]),
                "sf": np.ascontiguousarray(s_full[:, s]),
                "mf": np.ascontiguousarray(m_full[:, s]),
                "bs": np.ascontiguousarray(bsr[:, s]),
            }
        )
    return in_maps


def kernel(x, packed, mn, inv_scale, bias):
    from concourse.bass_utils import run_bass_kernel_spmd

    in_maps = host_prep(x, packed, mn, inv_scale, bias)
    nc = build_bass()
    res = run_bass_kernel_spmd(nc, in_maps, core_ids=list(range(N_CORES)))
    outs = [r["out"] for r in res.results]
    full = np.concatenate(outs, axis=1)[:, :O_FULL]
    return np.ascontiguousarray(full.astype(np.float32))


# revision 22
# speedup vs baseline: 95.6335x; 1.3253x over previous
"""CLinear (4-bit group-quantized linear) Trainium2 kernel.

y = x @ dequant(packed, inv_scale, mn).T + bias

Full shapes: x [4096, 4096] f32, packed [11008, 64, 32] u8,
mn/inv_scale [11008, 64, 1] f32, bias [11008] f32 -> out [4096, 11008] f32.

Strategy (column-parallel over out_features across 8 cores):
  - O padded 11008 -> 11264 = 8 * 1408; each core owns an o-slice of 1408.
  - Host does *layout only*: transpose x -> x^T (bf16, tiled so each
    token-tile's DMA reads are 8KB-contiguous per partition), re-packs the
    4-bit nibbles so the device unpacks directly into W^T [k, o] k-tiles,
    and transposes the per-group scale/offset tensors.
  - Device: for each 128-row k-tile, one fused DVE scalar_tensor_tensor
    (nibble-extract via bitwise_and, multiply by group scale) + one
    tensor_tensor add of the group offset produces a resident bf16
    W^T tile.  The hi-nibble path uses scale/16 so no shift is needed.
  - Matmul: out-tile [128 tokens, <=512 o] PSUM accumulated over 32
    k-tiles; lhsT = x^T tile (stationary), rhs = W^T tile.  Bias is
    added during the PSUM->SBUF copy (DVE tensor_tensor add with a
    partition-broadcast bias tile).
"""

from contextlib import ExitStack

import ml_dtypes
import numpy as np

O_FULL, I_DIM = 11008, 4096
TOKENS = 4096
GS = 64
NG = I_DIM // GS
N_CORES = 8
O_PAD = 11264  # 8 * 1408 = 88 * 128
O_C = O_PAD // N_CORES

BF16NP = ml_dtypes.bfloat16


def o_chunks_for(o_c, chunk=512):
    chunks = []
    off = 0
    while off < o_c:
        ln = min(chunk, o_c - off)
        chunks.append((off, ln))
        off += ln
    return chunks


def build_bass(T=TOKENS, K=I_DIM, o_c=O_C, chunk=512, n_devices=N_CORES, reps=1):
    import concourse.bass as bass
    import concourse.tile as tile
    from concourse import bacc, mybir

    BF16, F32, U8 = mybir.dt.bfloat16, mybir.dt.float32, mybir.dt.uint8
    Alu = mybir.AluOpType

    nkt = K // 128   # k-tiles of 128
    nbt = K // 256   # packed byte tiles (each yields 2 k-tiles)
    nti = T // 128   # token tiles
    ngc = K // GS    # groups along K
    chunks = o_chunks_for(o_c, chunk)

    nc = bacc.Bacc(
        "TRN2",
        target_bir_lowering=False,
        debug=False,
        enable_asserts=False,
        num_devices=n_devices,
    )
    xt = nc.dram_tensor("xt", [T, K], BF16, kind="ExternalInput").ap()
    pk = nc.dram_tensor("pk", [K // 2, o_c], U8, kind="ExternalInput").ap()
    # scale/offset pre-expanded on host to one row per k (scale/16 on hi k-tiles)
    sf = nc.dram_tensor("sf", [K, o_c], F32, kind="ExternalInput").ap()
    mf = nc.dram_tensor("mf", [K, o_c], F32, kind="ExternalInput").ap()
    bs = nc.dram_tensor("bs", [1, o_c], F32, kind="ExternalInput").ap()    # bias
    out = nc.dram_tensor("out", [T, o_c], F32, kind="ExternalOutput").ap()

    with ExitStack() as ctx:
        tc = ctx.enter_context(tile.TileContext(nc))
        wpool = ctx.enter_context(tc.tile_pool(name="wpool", bufs=nkt))
        npool = ctx.enter_context(tc.tile_pool(name="npool", bufs=nkt))
        bpool = ctx.enter_context(tc.tile_pool(name="bpool", bufs=3))
        spool = ctx.enter_context(tc.tile_pool(name="spool", bufs=3))
        tpool = ctx.enter_context(tc.tile_pool(name="tpool", bufs=2))
        xpool = ctx.enter_context(tc.tile_pool(name="xpool", bufs=3))
        pspool = ctx.enter_context(tc.tile_pool(name="pspool", bufs=8, space="PSUM"))
        opool = ctx.enter_context(tc.tile_pool(name="opool", bufs=6))
        singles = ctx.enter_context(tc.tile_pool(name="singles", bufs=1))

        # bias broadcast to all 128 partitions, once
        bias_sb = singles.tile([128, o_c], F32)
        nc.gpsimd.dma_start(
            out=bias_sb,
            in_=bass.AP(tensor=bs.tensor, offset=0, ap=[[0, 128], [1, o_c]]),
        )

        for rep in range(reps):
            # ---- unpack: resident u8 nibble k-tiles (GPSIMD, full width) ----
            nibs = []
            for bt in range(nbt):
                pkt = bpool.tile([128, o_c], U8, tag="pkt")
                nc.sync.dma_start(out=pkt, in_=pk[bt * 128 : (bt + 1) * 128, :])
                for half in range(2):
                    msk = 0xF0 if half == 0 else 0x0F
                    nib = npool.tile([128, o_c], U8, name=f"nib{2*bt+half}", tag="nib")
                    nc.vector.tensor_scalar(
                        out=nib, in0=pkt, scalar1=msk, scalar2=None, op0=Alu.bitwise_and
                    )
                    nibs.append(nib)

            # ---- dequant chunk-major so the first o-chunk's W columns are
            # ready for all k-tiles early (PE starts without waiting on the
            # whole dequant) ----
            wtiles = [wpool.tile([128, o_c], BF16, name=f"w{kt}", tag="w")
                      for kt in range(nkt)]
            for o_off, o_len in chunks:
                for kt in range(nkt):
                    osl = slice(o_off, o_off + o_len)
                    s_exp = spool.tile([128, o_len], F32, tag="s_exp")
                    m_exp = spool.tile([128, o_len], F32, tag="m_exp")
                    nc.sync.dma_start(
                        out=s_exp, in_=sf[kt * 128 : (kt + 1) * 128, osl]
                    )
                    nc.sync.dma_start(
                        out=m_exp, in_=mf[kt * 128 : (kt + 1) * 128, osl]
                    )
                    tmp = tpool.tile([128, o_len], F32, tag="tmp")
                    nc.vector.tensor_tensor(
                        out=tmp, in0=nibs[kt][:, osl], in1=s_exp, op=Alu.mult
                    )
                    nc.vector.tensor_add(wtiles[kt][:, osl], tmp, m_exp)

            # ---- matmul: out[ti, oc] = sum_kt xT[kt, ti].T @ WT[kt][:, oc] ----
            for ti in range(nti):
                xt_t = xpool.tile([128, nkt, 128], BF16, tag="xt_t")
                nc.sync.dma_start(out=xt_t, in_=xt[ti * 128 : (ti + 1) * 128, :])
                pss = []
                for o_off, o_len in chunks:
                    ps = pspool.tile([128, o_len], F32, tag="ps")
                    pss.append((ps, o_off, o_len))
                for kt in range(nkt):
                    for ps, o_off, o_len in pss:
                        nc.tensor.matmul(
                            ps,
                            lhsT=xt_t[:, kt, :],
                            rhs=wtiles[kt][:, o_off : o_off + o_len],
                            start=(kt == 0),
                            stop=(kt == nkt - 1),
                        )
                for ps, o_off, o_len in pss:
                    osb = opool.tile([128, o_len], F32, tag="osb")
                    nc.vector.tensor_add(osb, ps, bias_sb[:, o_off : o_off + o_len])
                    nc.sync.dma_start(
                        out=out[ti * 128 : (ti + 1) * 128, o_off : o_off + o_len],
                        in_=osb,
                    )

    nc.compile()
    return nc


def host_prep(x, packed, mn, inv_scale, bias, o_pad=O_PAD, n_cores=N_CORES):
    """Pure layout transforms; returns per-core input maps."""
    x = np.asarray(x, dtype=np.float32)
    packed = np.asarray(packed, dtype=np.uint8)
    mn = np.asarray(mn, dtype=np.float32)
    inv_scale = np.asarray(inv_scale, dtype=np.float32)
    bias = np.asarray(bias, dtype=np.float32)

    T, K = x.shape
    O = packed.shape[0]
    ng = packed.shape[1]
    o_c = o_pad // n_cores
    nkt, nti, nbt = K // 128, T // 128, K // 256

    # x^T, bf16, rearranged so token-tile ti occupies rows [128*ti, 128*(ti+1))
    # with per-partition-contiguous K (each DMA row is a full 8KB K-run).
    xT = x.T.astype(BF16NP)  # [K, T]
    xh = np.ascontiguousarray(
        xT.reshape(nkt, 128, nti, 128).transpose(2, 1, 0, 3).reshape(T, K)
    )
    # row (ti*128 + p), col (kt*128 + t) = xT[kt*128 + p, ti*128 + t]

    # unpack nibbles -> q [O, K], transpose, pad, re-pack so that byte tile bt
    # unpacks into k-tiles 2bt (hi nibble) and 2bt+1 (lo nibble).
    hi = packed >> 4
    lo = packed & 0x0F
    q = np.concatenate([hi, lo], axis=2).reshape(O, K)  # q[o, g*64+j]
    qT = np.zeros((K, o_pad), dtype=np.uint8)
    qT[:, :O] = q.T
    q4 = qT.reshape(nbt, 2, 128, o_pad)
    pk = ((q4[:, 0] << 4) | q4[:, 1]).reshape(K // 2, o_pad)

    sT = np.zeros((ng, o_pad), dtype=np.float32)
    sT[:, :O] = inv_scale[:, :, 0].T
    mT = np.zeros((ng, o_pad), dtype=np.float32)
    mT[:, :O] = mn[:, :, 0].T
    # expand to one row per k; hi k-tiles (even kt) get scale/16 because the
    # nibble arrives as (byte & 0xF0) = 16*q.
    s_full = np.repeat(sT, GS, axis=0)  # [K, o_pad], row k -> group k//64
    m_full = np.repeat(mT, GS, axis=0)
    ktile_of_k = np.arange(K) // 128
    s_full[ktile_of_k % 2 == 0] /= 16.0
    bsr = np.zeros((1, o_pad), dtype=np.float32)
    bsr[0, :O] = bias

    in_maps = []
    for c in range(n_cores):
        s = slice(c * o_c, (c + 1) * o_c)
        in_maps.append(
            {
                "xt": xh,
                "pk": np.ascontiguousarray(pk[:, s]),
                "sf": np.ascontiguousarray(s_full[:, s]),
                "mf": np.ascontiguousarray(m_full[:, s]),
                "bs": np.ascontiguousarray(bsr[:, s]),
            }
        )
    return in_maps


def kernel(x, packed, mn, inv_scale, bias):
    from concourse.bass_utils import run_bass_kernel_spmd

    in_maps = host_prep(x, packed, mn, inv_scale, bias)
    nc = build_bass()
    res = run_bass_kernel_spmd(nc, in_maps, core_ids=list(range(N_CORES)))
    outs = [r["out"] for r in res.results]
    full = np.concatenate(outs, axis=1)[:, :O_FULL]
    return np.ascontiguousarray(full.astype(np.float32))
